# revision 9
# baseline (speedup 1.0000x reference)
"""Trainium2 Bass kernel for nn_LogicConvUnfold.

Math: reference computes, per kernel k, windows a,b of x (gathered at
per-kernel (h,w,c) offsets) and a 16-term weighted sum of soft logic
gates over (a, b, ab).  Grouping terms by {1, a, b, ab} collapses it to

    out_k = Cab_k*a*b + Ca_k*a + Cb_k*b + C1_k

and, for |Cab| not tiny, further factors as

    out_k = (b + alpha)*(Cab*a + Cb) + gamma,
    alpha = Ca/Cab, gamma = C1 - Ca*Cb/Cab

so the per-kernel work is 3 ops spread over 3 engines:
    u  = Cab*a + Cb          tensor_scalar        (GPSIMD / Pool)
    w  = (b + alpha)*u       scalar_tensor_tensor (DVE, a few on Pool)
    res= Identity(w + gamma) activation, bf16 out (ACT)
Kernels with |Cab| < 1e-3 use a 3-op DVE fallback (ts, tt, stt) with
bias C1, avoiding the ill-conditioned division.

Sharding (8 cores): 2-way batch x 4-way kernel grid.  Core c handles
batches [4*(c%2), +4) and kernels [32*(c//2), +32).

Device layout: partition p = b_local*32 + iblk holds a 6-row halo slab
of all 8 channels of its batch: xp[b_local, :, 4*iblk : 4*iblk+6, :]
(x padded H 128->130 so the last block's halo is in bounds).  All
per-kernel window shifts (dh, dw in 0..2, channel select) become
free-dim offsets, identical across partitions: the gather is a
statically-offset strided view.  The slab is loaded by ONE DMA (halo
rows re-read from DRAM; the source AP is built raw on a flat tensor).

Output: ACT writes bf16 results into chunk tiles laid out exactly like
the flat DRAM output [128, NK*4*126]; chunks of 4 kernels are DMA'd as
fully contiguous 4KB-per-partition transfers.  The host upcasts to
fp32 and reshapes (final-rounding-only bf16: rel err ~4e-3 << 2e-2).

The program is SPMD (one NEFF for all 8 cores); per-core kernel sets
are selected by 4 Tile If-blocks guarded by a per-core input flag with
that quarter's 32 kernels' offsets and coefficients baked in as
immediates (the builder runs at call time, so any input still produces
a correct, freshly compiled, kernel).
"""

import contextlib
import sys

sys.path.insert(0, "/opt/trn_rl_repo")

import numpy as np

import concourse.bass as bass
import concourse.tile as tile
from concourse import bacc, mybir
from concourse.bass_utils import run_bass_kernel_spmd

B, C, H, W = 8, 8, 128, 128
K = 128
OH, OW = 126, 126
NB = 4   # batches per core
NK = 32  # kernels per core
L = 4    # output rows per block
NBLK = 32  # row blocks per batch
HP = H + 2  # padded rows
SLAB_F = C * 6 * W  # free elems per partition in the slab (6144)
FKP = L * OW        # elems per kernel per partition (504)
OUTF = NK * FKP     # flat output elems per partition (16128)
XPTOT = NB * C * HP * W  # flat padded input elems per core
CHUNK = 16  # kernels per output DMA
CAB_EPS = 1e-3  # below this, use the 3-op fallback path


def _coeffs(weights: np.ndarray) -> np.ndarray:
    """(K,16) weights -> (K,4) [Cab, Cb, Ca, C1], computed in f64."""
    w = weights.astype(np.float64)
    cab = (w[:, 1] - w[:, 2] - w[:, 4] - 2 * w[:, 6] - w[:, 7] + w[:, 8]
           + 2 * w[:, 9] + w[:, 11] + w[:, 13] - w[:, 14])
    ca = (w[:, 2] + w[:, 3] + w[:, 6] + w[:, 7] - w[:, 8] - w[:, 9]
          - w[:, 12] - w[:, 13])
    cb = (w[:, 4] + w[:, 5] + w[:, 6] + w[:, 7] - w[:, 8] - w[:, 9]
          - w[:, 10] - w[:, 11])
    c1 = w[:, 8:16].sum(axis=1)
    return np.stack([cab, cb, ca, c1], axis=1)


def _derived(cf: np.ndarray):
    """Per-kernel fast-path flag, alpha, gamma, final-bias (f64 in)."""
    cab, cb, ca, c1 = cf[:, 0], cf[:, 1], cf[:, 2], cf[:, 3]
    fast = np.abs(cab) >= CAB_EPS
    safe = np.where(fast, cab, 1.0)
    alpha = ca / safe
    gamma = c1 - ca * cb / safe
    bias = np.where(fast, gamma, c1)
    return fast, alpha, gamma, bias


def _build_program(cf, pa, pb, reps=1, loop_reps=False):
    fast, alpha, _, _ = _derived(cf)
    nc = bacc.Bacc("TRN2", debug=False, target_bir_lowering=False)
    xp_t = nc.dram_tensor("xp", (128, SLAB_F), mybir.dt.float32,
                          kind="ExternalInput")
    flags_t = nc.dram_tensor("flags", (1, 4), mybir.dt.int32,
                             kind="ExternalInput")
    gtab_t = nc.dram_tensor("gtab", (128, 2 * K), mybir.dt.float32,
                            kind="ExternalInput")
    out_t = nc.dram_tensor("out", (128, OUTF), mybir.dt.bfloat16,
                           kind="ExternalOutput")
    if loop_reps:
        nrep_t = nc.dram_tensor("nrep", (1, 1), mybir.dt.int32,
                                kind="ExternalInput")

    mult, add = mybir.AluOpType.mult, mybir.AluOpType.add

    with tile.TileContext(nc) as tc:
        with (
            tc.tile_pool(name="const", bufs=1) as cpool,
            tc.tile_pool(name="slabp", bufs=2) as spool,
            tc.tile_pool(name="work", bufs=8) as wpool,
            tc.tile_pool(name="outp", bufs=3) as opool,
        ):
          flags = cpool.tile([1, 4], mybir.dt.int32, tag="flags")
          nc.sync.dma_start(out=flags[:, :], in_=flags_t.ap()[:, :])
          gtab = cpool.tile([128, 2 * K], mybir.dt.float32, tag="gtab")
          nc.sync.dma_start(out=gtab[:, :], in_=gtab_t.ap()[:, :])
          fvals = [
              nc.values_load(flags[0:1, q:q + 1], min_val=0, max_val=1,
                             skip_runtime_bounds_check=True)
              for q in range(4)
          ]
          if loop_reps:
            nrep_sb = cpool.tile([1, 1], mybir.dt.int32, tag="nrep")
            nc.sync.dma_start(out=nrep_sb[:, :], in_=nrep_t.ap()[:, :])
            nval = nc.values_load(nrep_sb[0:1, 0:1], min_val=0,
                                  max_val=100000,
                                  skip_runtime_bounds_check=True)
            # Body holds 2 unrolled reps (slab/out tiles alternate pool
            # buffers at trace time -> cross-iteration double buffering),
            # so the loop steps by 2.  nrep must be even.
            rep_ctx = tc.For_i(0, nval, 2, hint_engines=(
                mybir.EngineType.DVE, mybir.EngineType.Activation,
                mybir.EngineType.SP, mybir.EngineType.Pool))
            body_reps = 2
          else:
            rep_ctx = contextlib.nullcontext()
            body_reps = reps
          with rep_ctx:
           for _rep in range(body_reps):
            slab = spool.tile([128, SLAB_F], mybir.dt.float32, tag="slab")
            # The host pre-tiles x into slab layout (halo duplicated), so
            # the load is ONE fully contiguous 128 x 24KB DMA.
            nc.sync.dma_start(out=slab[:, :], in_=xp_t.ap()[:, :])
            slab3 = slab[:, :].rearrange("p (r w) -> p r w", w=W)

            for q in range(4):
             with tc.If(fvals[q] > 0):
              och = None
              for kl in range(NK):
                k = 32 * q + kl
                ha, wa, ca = int(pa[k, 0]), int(pa[k, 1]), int(pa[k, 2])
                hb, wb, cb = int(pb[k, 0]), int(pb[k, 1]), int(pb[k, 2])
                cab_, cb_ = float(cf[k, 0]), float(cf[k, 1])
                ca_ = float(cf[k, 2])
                ra, rb = ca * 6 + ha, cb * 6 + hb
                av = slab3[:, ra:ra + L, wa:][:, :, :OW]
                bv = slab3[:, rb:rb + L, wb:][:, :, :OW]

                wv = wpool.tile([128, FKP], mybir.dt.float32, tag="w")
                w3 = wv[:, :].rearrange("p (i j) -> p i j", j=OW)
                if fast[k]:
                    uv = wpool.tile([128, FKP], mybir.dt.float32, tag="u")
                    u3 = uv[:, :].rearrange("p (i j) -> p i j", j=OW)
                    # Balance u = Cab*a + Cb across ACT (14+2) and DVE (16).
                    ts_act = (kl % 2 == 1 and kl < 28)
                    if ts_act:
                        nc.scalar.activation(
                            u3, av, mybir.ActivationFunctionType.Identity,
                            bias=gtab[:, K + k:K + k + 1], scale=cab_)
                    else:
                        nc.vector.tensor_scalar(
                            u3, av, cab_, cb_, op0=mult, op1=add)
                    if kl % 2 == 1:
                        # Pool route: v = b + alpha (DVE), w = u*v (GPSIMD).
                        vv = wpool.tile([128, FKP], mybir.dt.float32,
                                        tag="v")
                        v3 = vv[:, :].rearrange("p (i j) -> p i j", j=OW)
                        nc.vector.tensor_scalar(
                            v3, bv, float(alpha[k]), None, op0=add)
                        nc.gpsimd.tensor_tensor(w3, u3, v3, op=mult)
                    else:
                        nc.vector.scalar_tensor_tensor(
                            w3, bv, float(alpha[k]), u3, op0=add, op1=mult)
                else:
                    pv = wpool.tile([128, FKP], mybir.dt.float32, tag="u")
                    p3 = pv[:, :].rearrange("p (i j) -> p i j", j=OW)
                    nc.vector.tensor_scalar(
                        p3, av, cab_, cb_, op0=mult, op1=add)
                    mv = wpool.tile([128, FKP], mybir.dt.float32, tag="m")
                    m3 = mv[:, :].rearrange("p (i j) -> p i j", j=OW)
                    nc.vector.tensor_tensor(m3, p3, bv, op=mult)
                    nc.vector.scalar_tensor_tensor(
                        w3, av, ca_, m3, op0=mult, op1=add)

                ci = kl % CHUNK
                if ci == 0:
                    och = opool.tile([128, CHUNK * FKP], mybir.dt.bfloat16,
                                     tag="och")
                nc.scalar.activation(
                    och[:, ci * FKP:(ci + 1) * FKP], wv[:, :],
                    mybir.ActivationFunctionType.Identity,
                    bias=gtab[:, k:k + 1], scale=1.0,
                )
                if ci == CHUNK - 1:
                    c0 = (kl - CHUNK + 1) * FKP
                    nc.sync.dma_start(
                        out=out_t.ap()[:, c0:c0 + CHUNK * FKP],
                        in_=och[:, :])
    nc.compile()
    return nc


def _prep_inputs(x, weights, pairs_a, pairs_b):
    cf = _coeffs(np.asarray(weights))
    _, _, _, bias = _derived(cf)
    row = np.concatenate([bias, cf[:, 1]]).astype(np.float32)
    gtab = np.broadcast_to(row[None, :], (128, 2 * K)).copy()
    xpad = np.zeros((B, C, HP, W), dtype=np.float32)
    xpad[:, :, :H, :] = np.asarray(x)
    rows = (4 * np.arange(NBLK)[:, None] + np.arange(6)[None, :])  # (32,6)
    in_maps = []
    for core in range(8):
        bh, kq = core % 2, core // 2
        xc = xpad[4 * bh:4 * bh + 4]          # (NB, C, HP, W)
        xs = xc[:, :, rows, :]                # (NB, C, 32, 6, W)
        xs = xs.transpose(0, 2, 1, 3, 4)      # (NB, 32, C, 6, W)
        xp = np.ascontiguousarray(xs.reshape(128, SLAB_F))
        in_maps.append({
            "xp": xp,
            "flags": np.array([[1 if q == kq else 0 for q in range(4)]],
                              dtype=np.int32),
            "gtab": gtab,
        })
    return in_maps


def _assemble(results):
    full = np.empty((B, K, OH, OW), dtype=np.float32)
    for core in range(8):
        bh, kq = core % 2, core // 2
        o = np.asarray(results[core]["out"]).astype(np.float32)
        o = o.reshape(NB, NBLK, NK, L, OW).transpose(0, 2, 1, 3, 4)
        o = o.reshape(NB, NK, NBLK * L, OW)
        full[4 * bh:4 * bh + 4, 32 * kq:32 * kq + 32] = o[:, :, :OH, :]
    return full


def _run(inputs, trace=False):
    cf = _coeffs(np.asarray(inputs["weights"]))
    pa = np.asarray(inputs["pairs_a"])
    pb = np.asarray(inputs["pairs_b"])
    nc = _build_program(cf, pa, pb)
    in_maps = _prep_inputs(inputs["x"], inputs["weights"], pa, pb)
    r = run_bass_kernel_spmd(nc, in_maps, core_ids=list(range(8)),
                             trace=trace)
    return _assemble(r.results), r


def kernel(**inputs) -> np.ndarray:
    out, _ = _run(inputs)
    return out


# revision 10
# speedup vs baseline: 2.0490x; 2.0490x over previous
"""Trainium2 Bass kernel for nn_LogicConvUnfold.

Math: reference computes, per kernel k, windows a,b of x (gathered at
per-kernel (h,w,c) offsets) and a 16-term weighted sum of soft logic
gates over (a, b, ab).  Grouping terms by {1, a, b, ab} collapses it to

    out_k = Cab_k*a*b + Ca_k*a + Cb_k*b + C1_k

and, for |Cab| not tiny, further factors as

    out_k = (b + alpha)*(Cab*a + Cb) + gamma,
    alpha = Ca/Cab, gamma = C1 - Ca*Cb/Cab

so the per-kernel work is 3 ops spread over 3 engines:
    u  = Cab*a + Cb          tensor_scalar        (GPSIMD / Pool)
    w  = (b + alpha)*u       scalar_tensor_tensor (DVE, a few on Pool)
    res= Identity(w + gamma) activation, bf16 out (ACT)
Kernels with |Cab| < 1e-3 use a 3-op DVE fallback (ts, tt, stt) with
bias C1, avoiding the ill-conditioned division.

Sharding (8 cores): 2-way batch x 4-way kernel grid.  Core c handles
batches [4*(c%2), +4) and kernels [32*(c//2), +32).

Device layout: partition p = b_local*32 + iblk holds a 6-row halo slab
of all 8 channels of its batch: xp[b_local, :, 4*iblk : 4*iblk+6, :]
(x padded H 128->130 so the last block's halo is in bounds).  All
per-kernel window shifts (dh, dw in 0..2, channel select) become
free-dim offsets, identical across partitions: the gather is a
statically-offset strided view.  The slab is loaded by ONE DMA (halo
rows re-read from DRAM; the source AP is built raw on a flat tensor).

Output: ACT writes bf16 results into chunk tiles laid out exactly like
the flat DRAM output [128, NK*4*126]; chunks of 4 kernels are DMA'd as
fully contiguous 4KB-per-partition transfers.  The host upcasts to
fp32 and reshapes (final-rounding-only bf16: rel err ~4e-3 << 2e-2).

The program is SPMD (one NEFF for all 8 cores); per-core kernel sets
are selected by 4 Tile If-blocks guarded by a per-core input flag with
that quarter's 32 kernels' offsets and coefficients baked in as
immediates (the builder runs at call time, so any input still produces
a correct, freshly compiled, kernel).
"""

import contextlib
import sys

sys.path.insert(0, "/opt/trn_rl_repo")

import numpy as np

import concourse.bass as bass
import concourse.tile as tile
from concourse import bacc, mybir
from concourse.bass_utils import run_bass_kernel_spmd

B, C, H, W = 8, 8, 128, 128
K = 128
OH, OW = 126, 126
NB = 4   # batches per core
NK = 32  # kernels per core
L = 4    # output rows per block
NBLK = 32  # row blocks per batch
HP = H + 2  # padded rows
SLAB_F = C * 6 * W  # free elems per partition in the slab (6144)
FKP = L * OW        # elems per kernel per partition (504)
OUTF = NK * FKP     # flat output elems per partition (16128)
XPTOT = NB * C * HP * W  # flat padded input elems per core
CHUNK = 16  # kernels per output DMA
CAB_EPS = 1e-3  # below this, use the 3-op fallback path


def _coeffs(weights: np.ndarray) -> np.ndarray:
    """(K,16) weights -> (K,4) [Cab, Cb, Ca, C1], computed in f64."""
    w = weights.astype(np.float64)
    cab = (w[:, 1] - w[:, 2] - w[:, 4] - 2 * w[:, 6] - w[:, 7] + w[:, 8]
           + 2 * w[:, 9] + w[:, 11] + w[:, 13] - w[:, 14])
    ca = (w[:, 2] + w[:, 3] + w[:, 6] + w[:, 7] - w[:, 8] - w[:, 9]
          - w[:, 12] - w[:, 13])
    cb = (w[:, 4] + w[:, 5] + w[:, 6] + w[:, 7] - w[:, 8] - w[:, 9]
          - w[:, 10] - w[:, 11])
    c1 = w[:, 8:16].sum(axis=1)
    return np.stack([cab, cb, ca, c1], axis=1)


def _derived(cf: np.ndarray):
    """Per-kernel fast-path flag, alpha, gamma, final-bias (f64 in)."""
    cab, cb, ca, c1 = cf[:, 0], cf[:, 1], cf[:, 2], cf[:, 3]
    fast = np.abs(cab) >= CAB_EPS
    safe = np.where(fast, cab, 1.0)
    alpha = ca / safe
    gamma = c1 - ca * cb / safe
    bias = np.where(fast, gamma, c1)
    return fast, alpha, gamma, bias


def _build_program(cf, pa, pb, reps=1, loop_reps=False):
    fast, alpha, _, _ = _derived(cf)
    nc = bacc.Bacc("TRN2", debug=False, target_bir_lowering=False)
    xp_t = nc.dram_tensor("xp", (128, SLAB_F), mybir.dt.float32,
                          kind="ExternalInput")
    flags_t = nc.dram_tensor("flags", (1, 4), mybir.dt.int32,
                             kind="ExternalInput")
    gtab_t = nc.dram_tensor("gtab", (128, 2 * K), mybir.dt.float32,
                            kind="ExternalInput")
    out_t = nc.dram_tensor("out", (128, OUTF), mybir.dt.bfloat16,
                           kind="ExternalOutput")
    if loop_reps:
        nrep_t = nc.dram_tensor("nrep", (1, 1), mybir.dt.int32,
                                kind="ExternalInput")

    mult, add = mybir.AluOpType.mult, mybir.AluOpType.add

    with tile.TileContext(nc) as tc:
        with (
            tc.tile_pool(name="const", bufs=1) as cpool,
            tc.tile_pool(name="slabp", bufs=2) as spool,
            tc.tile_pool(name="work", bufs=8) as wpool,
            tc.tile_pool(name="outp", bufs=3) as opool,
        ):
          flags = cpool.tile([1, 4], mybir.dt.int32, tag="flags")
          nc.sync.dma_start(out=flags[:, :], in_=flags_t.ap()[:, :])
          gtab = cpool.tile([128, 2 * K], mybir.dt.float32, tag="gtab")
          nc.sync.dma_start(out=gtab[:, :], in_=gtab_t.ap()[:, :])
          fvals = [
              nc.values_load(flags[0:1, q:q + 1], min_val=0, max_val=1,
                             skip_runtime_bounds_check=True)
              for q in range(4)
          ]
          if loop_reps:
            nrep_sb = cpool.tile([1, 1], mybir.dt.int32, tag="nrep")
            nc.sync.dma_start(out=nrep_sb[:, :], in_=nrep_t.ap()[:, :])
            nval = nc.values_load(nrep_sb[0:1, 0:1], min_val=0,
                                  max_val=100000,
                                  skip_runtime_bounds_check=True)
            # Body holds 2 unrolled reps (slab/out tiles alternate pool
            # buffers at trace time -> cross-iteration double buffering),
            # so the loop steps by 2.  nrep must be even.
            rep_ctx = tc.For_i(0, nval, 2, hint_engines=(
                mybir.EngineType.DVE, mybir.EngineType.Activation,
                mybir.EngineType.SP, mybir.EngineType.Pool))
            body_reps = 2
          else:
            rep_ctx = contextlib.nullcontext()
            body_reps = reps
          with rep_ctx:
           for _rep in range(body_reps):
            slab = spool.tile([128, SLAB_F], mybir.dt.float32, tag="slab")
            # The host pre-tiles x into slab layout (halo duplicated), so
            # the load is ONE fully contiguous 128 x 24KB DMA.
            nc.sync.dma_start(out=slab[:, :], in_=xp_t.ap()[:, :])
            slab3 = slab[:, :].rearrange("p (r w) -> p r w", w=W)

            for q in range(4):
             with tc.If(fvals[q] > 0):
              och = None
              for kl in range(NK):
                k = 32 * q + kl
                ha, wa, ca = int(pa[k, 0]), int(pa[k, 1]), int(pa[k, 2])
                hb, wb, cb = int(pb[k, 0]), int(pb[k, 1]), int(pb[k, 2])
                cab_, cb_ = float(cf[k, 0]), float(cf[k, 1])
                ca_ = float(cf[k, 2])
                ra, rb = ca * 6 + ha, cb * 6 + hb
                av = slab3[:, ra:ra + L, wa:][:, :, :OW]
                bv = slab3[:, rb:rb + L, wb:][:, :, :OW]

                wv = wpool.tile([128, FKP], mybir.dt.float32, tag="w")
                w3 = wv[:, :].rearrange("p (i j) -> p i j", j=OW)
                if fast[k]:
                    uv = wpool.tile([128, FKP], mybir.dt.float32, tag="u")
                    u3 = uv[:, :].rearrange("p (i j) -> p i j", j=OW)
                    if kl % 16 in (0, 1, 2, 4, 6, 8, 10, 12, 14):  # 18/32 ts on ACT
                        nc.scalar.activation(
                            u3, av, mybir.ActivationFunctionType.Identity,
                            bias=gtab[:, K + k:K + k + 1], scale=cab_)
                    else:
                        nc.vector.tensor_scalar(
                            u3, av, cab_, cb_, op0=mult, op1=add)
                    nc.vector.scalar_tensor_tensor(
                        w3, bv, float(alpha[k]), u3, op0=add, op1=mult)
                else:
                    pv = wpool.tile([128, FKP], mybir.dt.float32, tag="u")
                    p3 = pv[:, :].rearrange("p (i j) -> p i j", j=OW)
                    nc.vector.tensor_scalar(
                        p3, av, cab_, cb_, op0=mult, op1=add)
                    mv = wpool.tile([128, FKP], mybir.dt.float32, tag="m")
                    m3 = mv[:, :].rearrange("p (i j) -> p i j", j=OW)
                    nc.vector.tensor_tensor(m3, p3, bv, op=mult)
                    nc.vector.scalar_tensor_tensor(
                        w3, av, ca_, m3, op0=mult, op1=add)

                ci = kl % CHUNK
                if ci == 0:
                    och = opool.tile([128, CHUNK * FKP], mybir.dt.bfloat16,
                                     tag="och")
                nc.scalar.activation(
                    och[:, ci * FKP:(ci + 1) * FKP], wv[:, :],
                    mybir.ActivationFunctionType.Identity,
                    bias=gtab[:, k:k + 1], scale=1.0,
                )
                if ci == CHUNK - 1:
                    c0 = (kl - CHUNK + 1) * FKP
                    nc.sync.dma_start(
                        out=out_t.ap()[:, c0:c0 + CHUNK * FKP],
                        in_=och[:, :])
    nc.compile()
    return nc


def _prep_inputs(x, weights, pairs_a, pairs_b):
    cf = _coeffs(np.asarray(weights))
    _, _, _, bias = _derived(cf)
    row = np.concatenate([bias, cf[:, 1]]).astype(np.float32)
    gtab = np.broadcast_to(row[None, :], (128, 2 * K)).copy()
    xpad = np.zeros((B, C, HP, W), dtype=np.float32)
    xpad[:, :, :H, :] = np.asarray(x)
    rows = (4 * np.arange(NBLK)[:, None] + np.arange(6)[None, :])  # (32,6)
    in_maps = []
    for core in range(8):
        bh, kq = core % 2, core // 2
        xc = xpad[4 * bh:4 * bh + 4]          # (NB, C, HP, W)
        xs = xc[:, :, rows, :]                # (NB, C, 32, 6, W)
        xs = xs.transpose(0, 2, 1, 3, 4)      # (NB, 32, C, 6, W)
        xp = np.ascontiguousarray(xs.reshape(128, SLAB_F))
        in_maps.append({
            "xp": xp,
            "flags": np.array([[1 if q == kq else 0 for q in range(4)]],
                              dtype=np.int32),
            "gtab": gtab,
        })
    return in_maps


def _assemble(results):
    full = np.empty((B, K, OH, OW), dtype=np.float32)
    for core in range(8):
        bh, kq = core % 2, core // 2
        o = np.asarray(results[core]["out"]).astype(np.float32)
        o = o.reshape(NB, NBLK, NK, L, OW).transpose(0, 2, 1, 3, 4)
        o = o.reshape(NB, NK, NBLK * L, OW)
        full[4 * bh:4 * bh + 4, 32 * kq:32 * kq + 32] = o[:, :, :OH, :]
    return full


def _run(inputs, trace=False):
    cf = _coeffs(np.asarray(inputs["weights"]))
    pa = np.asarray(inputs["pairs_a"])
    pb = np.asarray(inputs["pairs_b"])
    nc = _build_program(cf, pa, pb)
    in_maps = _prep_inputs(inputs["x"], inputs["weights"], pa, pb)
    r = run_bass_kernel_spmd(nc, in_maps, core_ids=list(range(8)),
                             trace=trace)
    return _assemble(r.results), r


def kernel(**inputs) -> np.ndarray:
    out, _ = _run(inputs)
    return out
